# revision 1
# baseline (speedup 1.0000x reference)
"""Trainium2 Bass kernel for nn_Linear_act_sp (2:4 activation-sparse linear).

Math (reference):
    max_act = max|x| over rows            [in]
    max_w   = max|W| over out rows        [in]
    s       = sqrt(max_act / clip(max_w)) [in]
    x_sp    = top2-of-4-magnitude prune of (x / s)
    out     = x_sp @ (W * s).T

Key identity: (x/s * mask) * s == x * mask elementwise, so
    out = (x * mask) @ W.T
where mask depends on the ranking of |x/s| within each contiguous group of 4
along the `in` dimension.

Implementation (8 NeuronCores, data-parallel over rows of x):
  Launch A: per-core partial abs-max reductions of x (row shard) and W (row
            shard) -> [2, 4096] partial maxes per core. abs on ACT, max tree
            split DVE/GpSimd, partition reduction via PE transpose + DVE
            free-dim max-reduce.
  Host:     exact f32 combine + s, r = 1/s (bit-identical to the f32 ops the
            reference performs; max is exact, host numpy divide/sqrt are
            correctly-rounded f32 just like the CPU reference).
  Launch B: two row-groups of 4 tiles. Group 0 is masked (min/max-threshold
            top-2-of-4 on v = |x|*r; equals the reference top_k mask absent
            exact boundary ties -- verified for this generator), transposed
            on PE, then its matmuls stream W.T (f32r, full PE rate) while
            group 1's mask runs on DVE and its PE transposes are interleaved
            into the group-0 matmul stream so the PE never stalls.
"""

import numpy as np

import concourse.bacc as bacc
import concourse.tile as tile
from concourse import mybir
from concourse.bass_utils import run_bass_kernel_spmd

AluOpType = mybir.AluOpType
ACTF = mybir.ActivationFunctionType
I32 = mybir.dt.int32
ABS_MASK = 0x7FFFFFFF

N_CORES = 8
N_ROWS = 8192          # 4*2048
D_IN = 4096
D_OUT = 4096
ROWS_PER_CORE = N_ROWS // N_CORES      # 1024
WROWS_PER_CORE = D_OUT // N_CORES      # 512
P = 128
EPS = np.float32(1e-8)

F32 = mybir.dt.float32
F32R = mybir.dt.float32r

_cache = {}

# test.py introspection: list of BassKernelResults from the last kernel() call
last_results = []


def _build_stats():
    nc = bacc.Bacc("TRN2", target_bir_lowering=False, debug=False,
                   num_devices=N_CORES)
    xs = nc.dram_tensor("xs", [ROWS_PER_CORE, D_IN], F32, kind="ExternalInput")
    ws = nc.dram_tensor("ws", [WROWS_PER_CORE, D_IN], F32, kind="ExternalInput")
    ident = nc.dram_tensor("ident", [P, P], F32, kind="ExternalInput")
    mx = nc.dram_tensor("mx", [2, D_IN], F32, kind="ExternalOutput")

    XT = ROWS_PER_CORE // P   # 8
    WT_ = WROWS_PER_CORE // P  # 4
    KT = D_IN // P            # 32

    with tile.TileContext(nc) as tc:
        with tc.tile_pool(name="xin", bufs=XT) as xpool, \
             tc.tile_pool(name="win", bufs=WT_) as wpool, \
             tc.tile_pool(name="misc", bufs=1) as mpool, \
             tc.tile_pool(name="ps", bufs=4, space="PSUM") as pspool:
            id_t = mpool.tile([P, P], F32, tag="ident")
            nc.sync.dma_start(id_t[:], ident.ap()[:, :])

            def absmax_tree(dram, pool, nt, tag):
                ts_ = []
                for t in range(nt):
                    ti = pool.tile([P, D_IN], F32, tag=tag, name=f"{tag}{t}")
                    nc.sync.dma_start(ti[:], dram.ap()[t * P:(t + 1) * P, :])
                    nc.scalar.activation(ti[:], ti[:], ACTF.Abs)
                    ts_.append(ti)
                stride = 1
                while stride < nt:
                    for i in range(0, nt, 2 * stride):
                        nc.vector.tensor_tensor(ts_[i][:], ts_[i][:],
                                                ts_[i + stride][:],
                                                op=AluOpType.max)
                    stride *= 2
                return ts_[0]

            acc_x = absmax_tree(xs, xpool, XT, "xt")
            acc_w = absmax_tree(ws, wpool, WT_, "wt")

            # partition reduce via PE transpose + free-dim max reduce
            for row, acc in ((0, acc_x), (1, acc_w)):
                red = mpool.tile([P, KT], F32, tag=f"red{row}")
                for k in range(KT):
                    ps = pspool.tile([P, P], F32, tag="ps", name=f"ps{row}_{k}")
                    nc.tensor.transpose(ps[:], acc[:, k * P:(k + 1) * P],
                                        id_t[:])
                    nc.vector.tensor_reduce(red[:, k:k + 1], ps[:],
                                            axis=mybir.AxisListType.X,
                                            op=AluOpType.max)
                # mx[row, 128k + i] = red[i, k]
                dst = mx.ap()[row:row + 1, :].rearrange("o (k i) -> i (o k)",
                                                        i=P)
                nc.sync.dma_start(dst, red[:])
    nc.compile()
    return nc


def _build_main():
    nc = bacc.Bacc("TRN2", target_bir_lowering=False, debug=False,
                   num_devices=N_CORES)
    xs = nc.dram_tensor("xs", [ROWS_PER_CORE, D_IN], F32, kind="ExternalInput")
    wt_d = nc.dram_tensor("wt", [D_IN, D_OUT], F32R, kind="ExternalInput")
    rr = nc.dram_tensor("rr", [P, D_IN], F32, kind="ExternalInput")
    ident = nc.dram_tensor("ident", [P, P], F32, kind="ExternalInput")
    ys = nc.dram_tensor("ys", [ROWS_PER_CORE, D_OUT], F32, kind="ExternalOutput")

    NT = ROWS_PER_CORE // P        # 8 row tiles
    KT = D_IN // P                 # 32 contraction tiles
    OT = D_OUT // 512              # 8 output column tiles
    H = 2048                       # column-half width
    QH = H // 4
    NH = D_IN // H                 # 2 halves per row tile
    GRP = 4                        # row tiles per group

    with tile.TileContext(nc) as tc:
        with tc.tile_pool(name="const", bufs=1) as cpool, \
             tc.tile_pool(name="xmT", bufs=1) as xpool, \
             tc.tile_pool(name="p1x", bufs=2) as p1x, \
             tc.tile_pool(name="p1v", bufs=2) as p1v, \
             tc.tile_pool(name="p1t", bufs=4) as p1t, \
             tc.tile_pool(name="wts", bufs=8) as wpool, \
             tc.tile_pool(name="outs", bufs=3) as opool, \
             tc.tile_pool(name="psum", bufs=8, space="PSUM") as psum:
            r_rep = cpool.tile([P, D_IN], F32, tag="rrep")
            nc.sync.dma_start(r_rep[:], rr.ap()[:, :])
            id_t = cpool.tile([P, P], F32, tag="ident")
            nc.sync.dma_start(id_t[:], ident.ap()[:, :])
            # transposed masked activations, n-major layout: lhsT for (k, n)
            # lives at xmT[:, n*4096 + k*128 : +128]   (i on partitions)
            xmT = xpool.tile([P, NT * D_IN], F32R, tag="xmT")

            def mask_half(n, h):
                """DVE/ACT: compute xm for rows [128n,128n+128) cols half h.
                Returns the masked xt tile (caller transposes)."""
                c0 = h * H
                xt = p1x.tile([P, H], F32, tag="xt", name=f"xt{n}_{h}")
                nc.sync.dma_start(xt[:], xs.ap()[n * P:(n + 1) * P, c0:c0 + H])
                v = p1v.tile([P, H], F32, tag="v", name=f"v{n}_{h}")
                # v = |x| * r  (ACT abs exact; DVE mult IEEE f32)
                nc.scalar.activation(v[:], xt[:], ACTF.Abs)
                nc.vector.tensor_mul(v[:], v[:], r_rep[:, c0:c0 + H])
                v4 = v[:].rearrange("p (g m) -> p g m", m=4)
                x4 = xt[:].rearrange("p (g m) -> p g m", m=4)
                vq = [v4[:, :, j] for j in range(4)]
                xq = [x4[:, :, j] for j in range(4)]
                # threshold = 2nd largest of the 4 =
                #   max(min(max(a,b), max(c,d)), max(min(a,b), min(c,d)))
                t1 = p1t.tile([P, QH], F32, tag="tt", name=f"t1_{n}{h}")
                t2 = p1t.tile([P, QH], F32, tag="tt", name=f"t2_{n}{h}")
                t3 = p1t.tile([P, QH], F32, tag="tt", name=f"t3_{n}{h}")
                t4 = p1t.tile([P, QH], F32, tag="tt", name=f"t4_{n}{h}")
                nc.vector.tensor_max(t1[:], vq[0], vq[1])
                nc.vector.tensor_tensor(t2[:], vq[0], vq[1], op=AluOpType.min)
                nc.vector.tensor_max(t3[:], vq[2], vq[3])
                nc.vector.tensor_tensor(t4[:], vq[2], vq[3], op=AluOpType.min)
                nc.vector.tensor_max(t2[:], t2[:], t4[:])
                nc.vector.tensor_tensor(t1[:], t1[:], t3[:], op=AluOpType.min)
                thr = t1
                nc.vector.tensor_max(thr[:], thr[:], t2[:])
                for j in range(4):
                    m = p1t.tile([P, QH], F32, tag="tt", name=f"m{n}{h}_{j}")
                    nc.vector.tensor_tensor(m[:], vq[j], thr[:],
                                            op=AluOpType.is_ge)
                    nc.vector.tensor_tensor(xq[j], xq[j], m[:],
                                            op=AluOpType.mult)
                return xt

            def transpose_half(n, h, xt):
                """PE transpose masked half into xmT; ACT drains PSUM."""
                c0 = h * H
                for kb in range(H // 512):
                    ps = psum.tile([P, 512], F32, tag="ps",
                                    name=f"tp{n}_{h}_{kb}")
                    for j in range(4):
                        k = (c0 // P) + kb * 4 + j
                        nc.tensor.transpose(
                            ps[:, j * P:(j + 1) * P],
                            xt[:, kb * 512 + j * P:kb * 512 + (j + 1) * P],
                            id_t[:])
                    dst0 = n * D_IN + c0 + kb * 512
                    nc.scalar.activation(xmT[:, dst0:dst0 + 512], ps[:],
                                         ACTF.Copy)

            def matmul_group(g, extra=None):
                """Matmuls for row tiles [4g, 4g+4) streaming all of W.T.
                `extra` maps o-index -> callables emitted after that o block
                (interleaves next group's transposes into the PE stream).
                PSUM results are DMA'd straight to DRAM (no SBUF bounce)."""
                ns = range(g * GRP, (g + 1) * GRP)
                for o in range(OT):
                    psn = {n: psum.tile([P, 512], F32, tag="ps",
                                        name=f"psn{g}_{o}_{n}")
                           for n in ns}
                    for k in range(KT):
                        w_t = wpool.tile([P, 512], F32R, tag="wt",
                                         name=f"w{g}_{o}_{k}")
                        nc.sync.dma_start(
                            w_t[:],
                            wt_d.ap()[k * P:(k + 1) * P, o * 512:(o + 1) * 512])
                        for n in ns:
                            nc.tensor.matmul(
                                psn[n][:],
                                xmT[:, n * D_IN + k * P:n * D_IN + (k + 1) * P],
                                w_t[:],
                                start=(k == 0), stop=(k == KT - 1))
                    for n in ns:
                        ot = opool.tile([P, 512], F32, tag="ot",
                                        name=f"ot{g}_{o}_{n}")
                        nc.vector.tensor_copy(ot[:], psn[n][:])
                        nc.sync.dma_start(
                            ys.ap()[n * P:(n + 1) * P, o * 512:(o + 1) * 512],
                            ot[:])
                    if extra and o in extra:
                        for fn in extra[o]:
                            fn()

            # group 0: mask + transpose up front, h-outer so the o=0
            # k<16 matmuls unblock after only the first four half-masks
            for h in range(NH):
                for n in range(GRP):
                    xt = mask_half(n, h)
                    transpose_half(n, h, xt)
            # group 1 masks run on DVE during group-0 matmuls; its PE
            # transposes are interleaved after each group-0 o-pair block so
            # the PE reaches them only after the corresponding mask is done.
            g1_halves = [(n, h) for h in range(NH) for n in range(GRP, NT)]
            masked = {}
            for n, h in g1_halves:
                masked[(n, h)] = mask_half(n, h)
            extra = {}
            for idx, (n, h) in enumerate(g1_halves):
                extra.setdefault(idx, []).append(
                    lambda nh=(n, h): transpose_half(nh[0], nh[1], masked[nh]))
            matmul_group(0, extra)
            matmul_group(1)
    nc.compile()
    return nc


def _get(name):
    if name not in _cache:
        _cache[name] = _build_stats() if name == "stats" else _build_main()
    return _cache[name]


def kernel(x: np.ndarray, W: np.ndarray) -> np.ndarray:
    global last_results
    last_results = []
    bs, seq, d_in = x.shape
    xf = np.ascontiguousarray(x.reshape(-1, d_in), dtype=np.float32)
    W = np.asarray(W, dtype=np.float32)

    x_shards = [np.ascontiguousarray(xf[c * ROWS_PER_CORE:(c + 1) * ROWS_PER_CORE])
                for c in range(N_CORES)]
    w_shards = [np.ascontiguousarray(W[c * WROWS_PER_CORE:(c + 1) * WROWS_PER_CORE])
                for c in range(N_CORES)]
    ident = np.eye(P, dtype=np.float32)

    # ---- Launch A: partial abs-max reductions ----
    nc_a = _get("stats")
    in_a = [{"xs": x_shards[c], "ws": w_shards[c], "ident": ident}
            for c in range(N_CORES)]
    res_a = run_bass_kernel_spmd(nc_a, in_a, list(range(N_CORES)))
    last_results.append(res_a)
    mx = np.stack([res_a.results[c]["mx"] for c in range(N_CORES)])  # [8,2,4096]
    max_act = np.max(mx[:, 0, :], axis=0).astype(np.float32)
    max_w = np.max(mx[:, 1, :], axis=0).astype(np.float32)

    # exact f32 host glue (bit-identical to reference CPU f32 arithmetic)
    s = np.sqrt((max_act / np.clip(max_w, EPS, None)).astype(np.float32)
                ).astype(np.float32)
    r = (np.float32(1.0) / s).astype(np.float32)
    r_rep = np.ascontiguousarray(np.broadcast_to(r, (P, D_IN)), dtype=np.float32)

    # ---- Launch B: mask + matmul ----
    wt = np.ascontiguousarray(W.T)                      # [in, out]
    nc_b = _get("main")
    in_b = [{"xs": x_shards[c], "wt": wt, "rr": r_rep, "ident": ident}
            for c in range(N_CORES)]
    res_b = run_bass_kernel_spmd(nc_b, in_b, list(range(N_CORES)))
    last_results.append(res_b)

    out = np.concatenate([res_b.results[c]["ys"] for c in range(N_CORES)],
                         axis=0)
    return out.reshape(bs, seq, D_OUT)



# revision 4
# speedup vs baseline: 1.1061x; 1.1061x over previous
"""Trainium2 Bass kernel for nn_Linear_act_sp (2:4 activation-sparse linear).

Math (reference):
    max_act = max|x| over rows            [in]
    max_w   = max|W| over out rows        [in]
    s       = sqrt(max_act / clip(max_w)) [in]
    x_sp    = top2-of-4-magnitude prune of (x / s)
    out     = x_sp @ (W * s).T

Key identity: (x/s * mask) * s == x * mask elementwise, so
    out = (x * mask) @ W.T
where mask keeps the top-2 of |x/s| within each contiguous group of 4
along `in`.

Single-launch implementation (8 NeuronCores, data-parallel over rows):
  * Host permutes the contraction dim: in' = j*1024 + g  (orig 4g+j), for
    both x and W.T.  A 2:4 group then occupies the SAME partition in four
    k-tile column blocks of the transposed activation, 8192 apart -- the
    top-2 mask becomes pure elementwise DVE work in the transposed domain
    and r=1/s is a per-partition scalar.  Contraction order is irrelevant
    to the matmul.
  * Phase 0: DMA x row shard + a [4096, 512] W.T stat slice.  PE transposes
    x into lhsT layout (mask-independent!) while DVE reduces abs-max stats
    from the transposed tiles (free-dim reduce; no extra transposes).
  * ~32 KB AllReduce(max) combines per-core stats (TOPSP/SDMA; compute
    engines untouched), then r = sqrt(clip(max_w)/max_act) on device.
  * Mask runs per k0-chunk (ACT |x|*r with per-partition scale, DVE
    min/max threshold top-2) feeding the matmul stream, whose contraction
    order visits chunks as they are masked.  f32r matmuls at full PE rate,
    LDWEIGHTS hidden behind the N=512 stream.
"""

import numpy as np

import concourse.bacc as bacc
import concourse.tile as tile
from concourse import mybir
from concourse.bass_utils import run_bass_kernel_spmd

AluOpType = mybir.AluOpType
ACTF = mybir.ActivationFunctionType

N_CORES = 8
N_ROWS = 8192          # 4*2048
D_IN = 4096
D_OUT = 4096
ROWS_PER_CORE = N_ROWS // N_CORES      # 1024
P = 128
EPS = np.float32(1e-8)

F32 = mybir.dt.float32
F32R = mybir.dt.float32r

NT = ROWS_PER_CORE // P    # 8 row tiles
KT = D_IN // P             # 32 contraction tiles
OT = D_OUT // 512          # 8 output column blocks
# contraction order: chunk k0 yields masked k-tiles {k0, k0+8, k0+16, k0+24}
KSEQ = [k0 + 8 * j for k0 in range(8) for j in range(4)]

_cache = {}

# test.py introspection: list of BassKernelResults from the last kernel() call
last_results = []


def _build():
    nc = bacc.Bacc("TRN2", target_bir_lowering=False, debug=False,
                   num_devices=N_CORES)
    xs = nc.dram_tensor("xs", [ROWS_PER_CORE, D_IN], F32, kind="ExternalInput")
    wt_d = nc.dram_tensor("wt", [D_IN, D_OUT], F32R, kind="ExternalInput")
    ws_d = nc.dram_tensor("ws", [D_IN, 512], F32, kind="ExternalInput")
    ident = nc.dram_tensor("ident", [P, P], F32, kind="ExternalInput")
    ys = nc.dram_tensor("ys", [ROWS_PER_CORE, D_OUT], F32, kind="ExternalOutput")

    with tile.TileContext(nc) as tc:
        with tc.tile_pool(name="const", bufs=1) as cpool, \
             tc.tile_pool(name="xmT", bufs=1) as xpool, \
             tc.tile_pool(name="xin", bufs=2) as xin, \
             tc.tile_pool(name="wst", bufs=4) as wst, \
             tc.tile_pool(name="sml", bufs=4) as sml, \
             tc.tile_pool(name="vv", bufs=8) as vpool, \
             tc.tile_pool(name="tt", bufs=6) as tpool, \
             tc.tile_pool(name="wts", bufs=6) as wpool, \
             tc.tile_pool(name="outs", bufs=4) as opool, \
             tc.tile_pool(name="dram", bufs=2, space="DRAM") as dpool, \
             tc.tile_pool(name="psum", bufs=8, space="PSUM") as psum:
            id_t = cpool.tile([P, P], F32, tag="ident")
            nc.sync.dma_start(id_t[:], ident.ap()[:, :])
            # transposed activations, k-major: block (k, n) at
            # xmT[:, k*1024 + n*128 : +128]  (in' on partitions, rows free)
            xmT = xpool.tile([P, KT * ROWS_PER_CORE], F32R, tag="xmT")
            xmT3 = xmT[:].rearrange("p (k c) -> p k c", c=ROWS_PER_CORE)
            # local stats: cols 0..31 = max|x| per (p, k); 32..63 = max|W|
            stats = cpool.tile([P, 2 * KT], F32, tag="stats")

            # ---- phase 0: x transpose + stats, W stat slice reduce ----
            H = 2048
            for n in range(NT):
                for h in range(2):
                    xt = xin.tile([P, H], F32, tag="xt", name=f"xt{n}_{h}")
                    nc.sync.dma_start(
                        xt[:], xs.ap()[n * P:(n + 1) * P, h * H:(h + 1) * H])
                    for kb in range(4):
                        k0 = h * 16 + kb * 4
                        ps = psum.tile([P, 512], F32, tag="ps",
                                       name=f"tp{n}_{h}_{kb}")
                        for j in range(4):
                            c = kb * 512 + j * P
                            nc.tensor.transpose(ps[:, j * P:(j + 1) * P],
                                                xt[:, c:c + P], id_t[:])
                        dst = xmT3[:, k0:k0 + 4, n * P:(n + 1) * P]
                        nc.scalar.activation(
                            dst, ps[:].rearrange("p (j c) -> p j c", c=P),
                            ACTF.Copy)
                        if n == 0:
                            nc.vector.tensor_reduce(
                                stats[:, k0:k0 + 4], dst,
                                axis=mybir.AxisListType.X,
                                op=AluOpType.max, apply_absolute_value=True)
                        else:
                            tmp = sml.tile([P, 4], F32, tag="tmp",
                                           name=f"tm{n}_{h}_{kb}")
                            nc.vector.tensor_reduce(
                                tmp[:], dst, axis=mybir.AxisListType.X,
                                op=AluOpType.max, apply_absolute_value=True)
                            nc.vector.tensor_tensor(
                                stats[:, k0:k0 + 4], stats[:, k0:k0 + 4],
                                tmp[:], op=AluOpType.max)
            for k in range(KT):
                wt_t = wst.tile([P, 512], F32, tag="ws", name=f"ws{k}")
                nc.sync.dma_start(wt_t[:], ws_d.ap()[k * P:(k + 1) * P, :])
                nc.vector.tensor_reduce(
                    stats[:, KT + k:KT + k + 1], wt_t[:],
                    axis=mybir.AxisListType.X,
                    op=AluOpType.max, apply_absolute_value=True)

            # ---- AllReduce(max) of [128, 64] stats ----
            bi = dpool.tile([P, 2 * KT], F32, tag="bi")
            bo = dpool.tile([P, 2 * KT], F32, tag="bo")
            nc.gpsimd.dma_start(bi[:], stats[:])
            nc.gpsimd.collective_compute(
                "AllReduce", AluOpType.max,
                replica_groups=[list(range(N_CORES))],
                ins=[bi[:].opt()], outs=[bo[:].opt()])
            gstats = cpool.tile([P, 2 * KT], F32, tag="gstats")
            nc.gpsimd.dma_start(gstats[:], bo[:])

            # ---- r = sqrt(clip(max_w, eps) / max_act), per (p, k) ----
            inv = cpool.tile([P, KT], F32, tag="inv")
            nc.vector.reciprocal(inv[:], gstats[:, 0:KT])
            wc = cpool.tile([P, KT], F32, tag="wc")
            nc.vector.tensor_scalar_max(wc[:], gstats[:, KT:2 * KT],
                                        float(EPS))
            nc.vector.tensor_mul(wc[:], wc[:], inv[:])
            rr = cpool.tile([P, KT], F32, tag="rr")
            nc.scalar.activation(rr[:], wc[:], ACTF.Sqrt)

            # ---- mask: top-2 of |x|*r within each quad, in place ----
            for k0 in range(8):
                for rh in range(2):
                    c0 = rh * 512
                    v = []
                    for j in range(4):
                        k = 8 * j + k0
                        vt = vpool.tile([P, 512], F32, tag="v",
                                        name=f"v{k0}_{rh}_{j}")
                        nc.scalar.activation(
                            vt[:], xmT3[:, k, c0:c0 + 512], ACTF.Abs,
                            scale=rr[:, k:k + 1])
                        v.append(vt)
                    t1 = tpool.tile([P, 512], F32, tag="t", name=f"t1_{k0}{rh}")
                    t2 = tpool.tile([P, 512], F32, tag="t", name=f"t2_{k0}{rh}")
                    t3 = tpool.tile([P, 512], F32, tag="t", name=f"t3_{k0}{rh}")
                    t4 = tpool.tile([P, 512], F32, tag="t", name=f"t4_{k0}{rh}")
                    nc.vector.tensor_max(t1[:], v[0][:], v[1][:])
                    nc.vector.tensor_tensor(t2[:], v[0][:], v[1][:],
                                            op=AluOpType.min)
                    nc.vector.tensor_max(t3[:], v[2][:], v[3][:])
                    nc.vector.tensor_tensor(t4[:], v[2][:], v[3][:],
                                            op=AluOpType.min)
                    nc.vector.tensor_tensor(t1[:], t1[:], t3[:],
                                            op=AluOpType.min)
                    nc.vector.tensor_max(t2[:], t2[:], t4[:])
                    nc.vector.tensor_max(t1[:], t1[:], t2[:])  # threshold
                    for j in range(4):
                        k = 8 * j + k0
                        nc.vector.tensor_tensor(v[j][:], v[j][:], t1[:],
                                                op=AluOpType.is_ge)
                        nc.vector.tensor_tensor(
                            xmT3[:, k, c0:c0 + 512], xmT3[:, k, c0:c0 + 512],
                            v[j][:], op=AluOpType.mult)

            # ---- matmul stream: out = xmT.T @ W.T ----
            for o in range(OT):
                psn = {n: psum.tile([P, 512], F32, tag="ps",
                                    name=f"mm{o}_{n}")
                       for n in range(NT)}
                for ki, k in enumerate(KSEQ):
                    w_t = wpool.tile([P, 512], F32R, tag="wt",
                                     name=f"w{o}_{k}")
                    nc.sync.dma_start(
                        w_t[:],
                        wt_d.ap()[k * P:(k + 1) * P, o * 512:(o + 1) * 512])
                    for n in range(NT):
                        nc.tensor.matmul(
                            psn[n][:],
                            xmT3[:, k, n * P:(n + 1) * P],
                            w_t[:],
                            start=(ki == 0), stop=(ki == KT - 1))
                for n in range(NT):
                    ot = opool.tile([P, 512], F32, tag="ot",
                                    name=f"ot{o}_{n}")
                    nc.scalar.activation(ot[:], psn[n][:], ACTF.Copy)
                    nc.sync.dma_start(
                        ys.ap()[n * P:(n + 1) * P, o * 512:(o + 1) * 512],
                        ot[:])
    nc.compile()
    return nc


def _get():
    if "main" not in _cache:
        _cache["main"] = _build()
    return _cache["main"]


# contraction-dim permutation: new col j*1024+g holds orig col 4g+j
_PERM = np.arange(D_IN).reshape(D_IN // 4, 4).T.reshape(-1)


def kernel(x: np.ndarray, W: np.ndarray) -> np.ndarray:
    global last_results
    last_results = []
    bs, seq, d_in = x.shape
    xf = x.reshape(-1, d_in).astype(np.float32, copy=False)
    W = np.asarray(W, dtype=np.float32)

    xp = np.ascontiguousarray(xf[:, _PERM])
    wtp = np.ascontiguousarray(W.T[_PERM, :])
    ident = np.eye(P, dtype=np.float32)

    nc = _get()
    in_maps = []
    for c in range(N_CORES):
        in_maps.append({
            "xs": np.ascontiguousarray(
                xp[c * ROWS_PER_CORE:(c + 1) * ROWS_PER_CORE]),
            "wt": wtp,
            "ws": np.ascontiguousarray(wtp[:, c * 512:(c + 1) * 512]),
            "ident": ident,
        })
    res = run_bass_kernel_spmd(nc, in_maps, list(range(N_CORES)))
    last_results.append(res)

    out = np.concatenate([res.results[c]["ys"] for c in range(N_CORES)],
                         axis=0)
    return out.reshape(bs, seq, D_OUT)
